# revision 38
# baseline (speedup 1.0000x reference)
"""Trainium2 Bass kernel: grouped-experts SwiGLU MLP with mid-RMSNorm.

Expert-parallel across 8 NeuronCores: core e computes expert e's token
block (tokens are pre-sorted by expert).  Host gathers each expert's
rows into a zero-padded [C, D] buffer, ships transposed activations and
weights, and scatters the per-core outputs back to flat token order.

Per-core math (fp16 operands, fp32 PSUM accumulation):
    h1 = x @ w1^T ; h3 = x @ w3^T          # [C, F]
    h  = silu(h1) * h3
    h  = h * rsqrt(mean(h^2) + eps)        # RMSNorm (scale folded to out)
    out = (h * mid_w) @ w2^T               # mid_w folded into w2 on host

Schedule (all weights SBUF-resident, streamed in once):
  fb0 w1-sweep k-outer (paces the initial weight/x DMAs), then t-outer
  sweeps for fb0 w3, fb1 w1, fb1 w3.  h tiles are transposed via the
  DMA xbar (dma_start_transpose) so the PE only runs the 480 matmuls.
  Phase C (out = hT.T @ w2T, scaled by rstd) runs per-tile at the end.
"""

import sys

sys.path.insert(0, "/opt/trn_rl_repo")

import numpy as np
from contextlib import ExitStack

import os

import concourse.bass as bass
import concourse.tile as tile
from concourse import bacc, mybir

P = 128
T = 4096
D = 2048
F = 1024
E = 8
NB = 512  # matmul moving-dim block (one PSUM bank of fp32)
EPS = 1e-6
F32 = mybir.dt.float32
F16 = mybir.dt.float16
ACTF = mybir.ActivationFunctionType

_PROGRAM_CACHE: dict[int, object] = {}
LAST_RESULTS = None  # test harness reads per-core outputs from here


def _run(nc, in_maps):
    """Execute the compiled program on the 8 axon-tunneled cores.

    If KERNEL_NTFF_DIR is set, wrap the execute in the axon NTFF profile
    hook so device profiles land there (test harness use only).
    """
    from concourse import bass2jax

    ntff_dir = os.environ.get("KERNEL_NTFF_DIR")
    if ntff_dir:
        if "/root/.axon_site" not in sys.path:
            sys.path.insert(0, "/root/.axon_site")
        from trn_agent_boot.trn_boot import _ntff_profile_via_ctypes

        hook = _ntff_profile_via_ctypes("/opt/axon/libaxon_pjrt.so")
        ids = [
            int(x) for x in os.environ.get("KERNEL_NTFF_CORES", "0").split(",")
        ]
        if hook is not None:
            with hook(ntff_dir, ids):
                return bass2jax.run_bass_via_pjrt(nc, in_maps, n_cores=len(in_maps))
    return bass2jax.run_bass_via_pjrt(nc, in_maps, n_cores=len(in_maps))


def _build_program(C: int):
    """Build + compile the single-core SPMD program for C padded rows."""
    NT = C // P  # token tiles per core (<= 6: PSUM holds NT + 2 banks)
    KD = D // P  # 16 contraction chunks for mm1
    KF = F // P  # 8 contraction chunks for mm2
    FB = F // NB  # 2 f-blocks
    DB = D // NB  # 4 d-blocks

    nc = bacc.Bacc(
        "TRN2",
        target_bir_lowering=True,
        debug=False,
        enable_asserts=False,
        num_devices=E,
    )
    xT_d = nc.dram_tensor("xT", [D, C], F16, kind="ExternalInput").ap()
    w1_d = nc.dram_tensor("w1t", [D, F], F16, kind="ExternalInput").ap()
    w3_d = nc.dram_tensor("w3t", [D, F], F16, kind="ExternalInput").ap()
    w2_d = nc.dram_tensor("w2t", [F, D], F16, kind="ExternalInput").ap()
    out_d = nc.dram_tensor("out", [C, D], F16, kind="ExternalOutput").ap()

    with tile.TileContext(nc) as tc, ExitStack() as ctx:
        singles = ctx.enter_context(tc.tile_pool(name="singles", bufs=1))
        xpool = ctx.enter_context(tc.tile_pool(name="x", bufs=1))
        w1pool = ctx.enter_context(tc.tile_pool(name="w1", bufs=1))
        w3pool = ctx.enter_context(tc.tile_pool(name="w3", bufs=1))
        w2pool = ctx.enter_context(tc.tile_pool(name="w2", bufs=1))
        hpool = ctx.enter_context(tc.tile_pool(name="h", bufs=1))
        htpool = ctx.enter_context(tc.tile_pool(name="ht", bufs=1))
        spool = ctx.enter_context(tc.tile_pool(name="s", bufs=1))
        qpool = ctx.enter_context(tc.tile_pool(name="sq", bufs=2))
        opool = ctx.enter_context(tc.tile_pool(name="o", bufs=2))
        stat = ctx.enter_context(tc.tile_pool(name="stat", bufs=1))
        ps_a = ctx.enter_context(tc.tile_pool(name="psa", bufs=1, space="PSUM"))
        ps_o = ctx.enter_context(tc.tile_pool(name="pso", bufs=2, space="PSUM"))

        eps_t = singles.tile([P, 1], F32, name="epsT")
        nc.gpsimd.memset(eps_t[:], EPS)
        dmy = singles.tile([P, NB], F16, name="dmy")
        nc.vector.memset(dmy[:], 0.0)

        # ---- input DMAs, all issued upfront in strict need order.
        # x + w2 ride the HWDGE scalar queue; w1/w3 stream on the SWDGE
        # gpsimd queue (back-to-back, no ring stalls): w1 fb0, w3 fb0,
        # w1 fb1, w3 fb1.
        xt = xpool.tile([P, KD, C], F16)
        xT_r = xT_d.rearrange("(k p) c -> p k c", p=P)
        for q in range(KD // 2):
            ks = bass.ds(q * 2, 2)
            nc.scalar.dma_start(xt[:, ks, :], xT_r[:, ks, :])

        def xs(k, t):
            return xt[:, k, t * P : t * P + P]

        w1s = w1pool.tile([P, KD, F], F16)
        w3s = w3pool.tile([P, KD, F], F16)
        w1_r = w1_d.rearrange("(k p) f -> p k f", p=P)
        w3_r = w3_d.rearrange("(k p) f -> p k f", p=P)
        # one need-ordered FIFO on the gpsimd SWDGE queue for ALL weights:
        # w1 fb0, w3 fb0, w1 fb1, w3 fb1, then w2.  The scalar queue
        # carries only x, so the first sweep's inputs never compete with
        # later streams for HBM bandwidth.  (HWDGE queues ring-stall past
        # ~8 transfers; gpsimd streams back-to-back.)
        for ws, w_r in ((w1s, w1_r), (w3s, w3_r)):
            for q in range(KD // 2):
                ks = bass.ds(q * 2, 2)
                nc.gpsimd.dma_start(ws[:, ks, 0:NB], w_r[:, ks, 0:NB])
        for ws, w_r in ((w1s, w1_r), (w3s, w3_r)):
            for q in range(KD // 4):
                ks = bass.ds(q * 4, 4)
                nc.gpsimd.dma_start(
                    ws[:, ks, NB : 2 * NB], w_r[:, ks, NB : 2 * NB]
                )

        def wsl(wi, fb, k):
            ws = w1s if wi == 0 else w3s
            return ws[:, k, bass.ds(fb * NB, NB)]

        # warm up the PE p-state while the first x/w1 chunks are still in
        # flight: dependency-free dummy matmuls keep the array busy from
        # right after the framework preamble, so the real sweep starts at
        # full speed instead of ramping through the slow p-states.
        pdmy = ps_o.tile([P, NB], F32, tag="po", name="pdmy")
        for _ in range(16):
            nc.tensor.matmul(pdmy[:], dmy[:, 0:P], dmy[:], start=True, stop=True)

        w2s = w2pool.tile([P, KF, D], F16)
        w2_r = w2_d.rearrange("(k p) d -> p k d", p=P)
        for q in range(KF // 2):
            ks = bass.ds(q * 2, 2)
            nc.gpsimd.dma_start(w2s[:, ks, :], w2_r[:, ks, :])

        ssqa = stat.tile([P, NT], F32, name="ssqa")
        ssqb = stat.tile([P, NT], F32, name="ssqb")
        ssq = stat.tile([P, NT], F32, name="ssq")
        std = stat.tile([P, NT], F32, name="std")
        rstd = stat.tile([P, NT], F32, name="rstd")
        h = {
            t: hpool.tile([P, F], F16, tag=f"h{t}", name=f"h{t}")
            for t in range(NT)
        }
        ht = {
            t: htpool.tile([P, KF, P], F16, tag=f"ht{t}", name=f"ht{t}")
            for t in range(NT)
        }
        s = {}
        KQ = 4  # k-chunks per sweep quarter

        # ===== phase A: four sweeps (w1 fb0, w3 fb0, w1 fb1, w3 fb1).
        # Sweep 1 is k-outer so the PE consumes the just-arriving x/w1
        # chunks progressively; later sweeps have resident weights and run
        # t-outer so each tile's epilogue overlaps the next tile's matmuls.
        # Sweep 1 is k-outer so the PE consumes the just-arriving x/w1
        # chunks progressively; later sweeps have resident weights and run
        # t-outer so each tile's epilogue overlaps the next tile's matmuls.
        def sweep(wi, fb, epilogue, k_outer=False):
            ps = {}
            for t in range(NT):
                ps[t] = ps_a.tile(
                    [P, NB], F32, tag=f"pa{t}", name=f"ps_{wi}_{fb}_{t}"
                )
            if k_outer:
                for k in range(KD):
                    for t in range(NT):
                        nc.tensor.matmul(
                            ps[t][:],
                            xs(k, t),
                            wsl(wi, fb, k),
                            start=(k == 0),
                            stop=(k == KD - 1),
                        )
                for t in range(NT):
                    epilogue(t, ps[t])
            else:
                for t in range(NT):
                    for k in range(KD):
                        nc.tensor.matmul(
                            ps[t][:],
                            xs(k, t),
                            wsl(wi, fb, k),
                            start=(k == 0),
                            stop=(k == KD - 1),
                        )
                    epilogue(t, ps[t])

        def ep_silu1(t, ps):
            s[t] = spool.tile([P, NB], F32, tag=f"s{t}", name=f"s0_{t}")
            nc.scalar.activation(s[t][:], ps[:], ACTF.Silu)

        def ep_h0(t, ps):
            nc.vector.tensor_mul(h[t][:, 0:NB], s[t][:], ps[:])
            hsq = qpool.tile([P, NB], F16, tag="hsq", name=f"hsq0_{t}")
            nc.scalar.activation(
                hsq[:], h[t][:, 0:NB], ACTF.Square, accum_out=ssqa[:, t : t + 1]
            )
            nc.scalar.dma_start_transpose(
                ht[t][:, 0 : KF // 2, :], h[t][:, 0:NB]
            )

        def ep_silu2(t, ps):
            sn = spool.tile([P, NB], F32, tag=f"s{t}", name=f"s1_{t}")
            nc.scalar.activation(sn[:], ps[:], ACTF.Silu)
            s[t] = sn

        def ep_h1(t, ps):
            nc.vector.tensor_mul(h[t][:, NB : 2 * NB], s[t][:], ps[:])
            hsq = qpool.tile([P, NB], F16, tag="hsq", name=f"hsq1_{t}")
            nc.scalar.activation(
                hsq[:],
                h[t][:, NB : 2 * NB],
                ACTF.Square,
                accum_out=ssqb[:, t : t + 1],
            )
            nc.scalar.dma_start_transpose(
                ht[t][:, KF // 2 : KF, :], h[t][:, NB : 2 * NB]
            )
            nc.vector.tensor_add(
                ssq[:, t : t + 1], ssqa[:, t : t + 1], ssqb[:, t : t + 1]
            )
            nc.scalar.activation(
                std[:, t : t + 1],
                ssq[:, t : t + 1],
                ACTF.Sqrt,
                bias=eps_t[:],
                scale=1.0 / F,
            )
            nc.vector.reciprocal(rstd[:, t : t + 1], std[:, t : t + 1])

        sweep(0, 0, ep_silu1, k_outer=True)
        sweep(1, 0, ep_h0)
        sweep(0, 1, ep_silu2)
        sweep(1, 1, ep_h1)

        # ===== phase C: out = hT.T @ w2T, scaled by rstd.  db outer so the
        # w2 stream is consumed progressively; out DMAs per (t, db) chunk.
        for db in range(DB):
            dsl = bass.ds(db * NB, NB)
            for t in range(NT):
                pso = ps_o.tile([P, NB], F32, tag="po")
                for fc in range(KF):
                    nc.tensor.matmul(
                        pso[:],
                        ht[t][:, fc, :],
                        w2s[:, fc, dsl],
                        start=(fc == 0),
                        stop=(fc == KF - 1),
                    )
                ob = opool.tile([P, NB], F16, tag="ob", name=f"ob{t}_{db}")
                nc.vector.tensor_scalar_mul(ob[:], pso[:], rstd[:, t : t + 1])
                nc.sync.dma_start(out_d[t * P : (t + 1) * P, dsl], ob[:])

    nc.compile()
    return nc


def _get_program(C: int):
    if C not in _PROGRAM_CACHE:
        _PROGRAM_CACHE[C] = _build_program(C)
    return _PROGRAM_CACHE[C]


def kernel(x, w1, w2, w3, mid_w, num_tokens_per_expert):
    global LAST_RESULTS
    x = np.ascontiguousarray(np.asarray(x, dtype=np.float32))
    w1 = np.asarray(w1, dtype=np.float32)
    w2 = np.asarray(w2, dtype=np.float32)
    w3 = np.asarray(w3, dtype=np.float32)
    mid_w = np.asarray(mid_w, dtype=np.float32)
    counts = np.asarray(num_tokens_per_expert).astype(np.int64)

    T_, D_ = x.shape
    E_, F_, _ = w1.shape
    Ccap = (T_ // E_) * 3 // 2  # reference static capacity (768)
    ends = np.cumsum(counts)
    starts = ends - counts
    eff = np.minimum(np.maximum(counts, 0), Ccap)  # rows actually computed

    C = int(max(P, -(-int(eff.max()) // P) * P))  # pad to token-tile multiple
    nc = _get_program(C)

    in_maps = []
    for e in range(E_):
        cnt = int(eff[e])
        s = int(starts[e])
        xg = np.zeros((C, D_), np.float32)
        if cnt > 0:
            rows = np.clip(s + np.arange(cnt), 0, T_ - 1)
            xg[:cnt] = x[rows]
        in_maps.append(
            {
                "xT": np.ascontiguousarray(xg.T).astype(np.float16),
                "w1t": np.ascontiguousarray(w1[e].T).astype(np.float16),
                "w3t": np.ascontiguousarray(w3[e].T).astype(np.float16),
                "w2t": np.ascontiguousarray((w2[e] * mid_w[None, :]).T).astype(
                    np.float16
                ),
            }
        )

    LAST_RESULTS = _run(nc, in_maps)
    outs = [LAST_RESULTS[e]["out"] for e in range(E_)]

    # scatter back to flat token order, mirroring the reference's clamping
    tok = np.arange(T_)
    eid = np.clip(np.searchsorted(ends, tok, side="right"), 0, E_ - 1)
    pos = tok - starts[eid]
    idx = np.minimum(pos, Ccap - 1)
    valid = (idx >= 0) & (idx < eff[eid])
    idx_safe = np.clip(idx, 0, C - 1)
    stacked = np.stack(outs, axis=0)  # [E, C, D]
    result = stacked[eid, idx_safe].astype(np.float32)
    result[~valid] = 0.0
    return result


# revision 41
# speedup vs baseline: 1.0034x; 1.0034x over previous
"""Trainium2 Bass kernel: grouped-experts SwiGLU MLP with mid-RMSNorm.

Expert-parallel across 8 NeuronCores: core e computes expert e's token
block (tokens are pre-sorted by expert).  Host gathers each expert's
rows into a zero-padded [C, D] buffer, ships transposed activations and
weights, and scatters the per-core outputs back to flat token order.

Per-core math (fp16 operands, fp32 PSUM accumulation):
    h1 = x @ w1^T ; h3 = x @ w3^T          # [C, F]
    h  = silu(h1) * h3
    h  = h * rsqrt(mean(h^2) + eps)        # RMSNorm (scale folded to out)
    out = (h * mid_w) @ w2^T               # mid_w folded into w2 on host

Schedule (all weights SBUF-resident, streamed in once):
  fb0 w1-sweep k-outer (paces the initial weight/x DMAs), then t-outer
  sweeps for fb0 w3, fb1 w1, fb1 w3.  h tiles are transposed via the
  DMA xbar (dma_start_transpose) so the PE only runs the 480 matmuls.
  Phase C (out = hT.T @ w2T, scaled by rstd) runs per-tile at the end.
"""

import sys

sys.path.insert(0, "/opt/trn_rl_repo")

import numpy as np
from contextlib import ExitStack

import os

import concourse.bass as bass
import concourse.tile as tile
from concourse import bacc, mybir

P = 128
T = 4096
D = 2048
F = 1024
E = 8
NB = 512  # matmul moving-dim block (one PSUM bank of fp32)
EPS = 1e-6
F32 = mybir.dt.float32
F16 = mybir.dt.float16
ACTF = mybir.ActivationFunctionType

_PROGRAM_CACHE: dict[int, object] = {}
LAST_RESULTS = None  # test harness reads per-core outputs from here


def _run(nc, in_maps):
    """Execute the compiled program on the 8 axon-tunneled cores.

    If KERNEL_NTFF_DIR is set, wrap the execute in the axon NTFF profile
    hook so device profiles land there (test harness use only).
    """
    from concourse import bass2jax

    ntff_dir = os.environ.get("KERNEL_NTFF_DIR")
    if ntff_dir:
        if "/root/.axon_site" not in sys.path:
            sys.path.insert(0, "/root/.axon_site")
        from trn_agent_boot.trn_boot import _ntff_profile_via_ctypes

        hook = _ntff_profile_via_ctypes("/opt/axon/libaxon_pjrt.so")
        ids = [
            int(x) for x in os.environ.get("KERNEL_NTFF_CORES", "0").split(",")
        ]
        if hook is not None:
            with hook(ntff_dir, ids):
                return bass2jax.run_bass_via_pjrt(nc, in_maps, n_cores=len(in_maps))
    return bass2jax.run_bass_via_pjrt(nc, in_maps, n_cores=len(in_maps))


def _build_program(C: int):
    """Build + compile the single-core SPMD program for C padded rows."""
    NT = C // P  # token tiles per core (<= 6: PSUM holds NT + 2 banks)
    KD = D // P  # 16 contraction chunks for mm1
    KF = F // P  # 8 contraction chunks for mm2
    FB = F // NB  # 2 f-blocks
    DB = D // NB  # 4 d-blocks

    nc = bacc.Bacc(
        "TRN2",
        target_bir_lowering=False,
        debug=False,
        enable_asserts=False,
        num_devices=E,
    )
    xT_d = nc.dram_tensor("xT", [D, C], F16, kind="ExternalInput").ap()
    w1_d = nc.dram_tensor("w1t", [D, F], F16, kind="ExternalInput").ap()
    w3_d = nc.dram_tensor("w3t", [D, F], F16, kind="ExternalInput").ap()
    w2_d = nc.dram_tensor("w2t", [F, D], F16, kind="ExternalInput").ap()
    out_d = nc.dram_tensor("out", [C, D], F16, kind="ExternalOutput").ap()

    with tile.TileContext(nc) as tc, ExitStack() as ctx:
        singles = ctx.enter_context(tc.tile_pool(name="singles", bufs=1))
        xpool = ctx.enter_context(tc.tile_pool(name="x", bufs=1))
        w1pool = ctx.enter_context(tc.tile_pool(name="w1", bufs=1))
        w3pool = ctx.enter_context(tc.tile_pool(name="w3", bufs=1))
        w2pool = ctx.enter_context(tc.tile_pool(name="w2", bufs=1))
        hpool = ctx.enter_context(tc.tile_pool(name="h", bufs=1))
        htpool = ctx.enter_context(tc.tile_pool(name="ht", bufs=1))
        spool = ctx.enter_context(tc.tile_pool(name="s", bufs=1))
        qpool = ctx.enter_context(tc.tile_pool(name="sq", bufs=2))
        opool = ctx.enter_context(tc.tile_pool(name="o", bufs=2))
        stat = ctx.enter_context(tc.tile_pool(name="stat", bufs=1))
        ps_a = ctx.enter_context(tc.tile_pool(name="psa", bufs=1, space="PSUM"))
        ps_o = ctx.enter_context(tc.tile_pool(name="pso", bufs=2, space="PSUM"))

        eps_t = singles.tile([P, 1], F32, name="epsT")
        nc.gpsimd.memset(eps_t[:], EPS)
        dmy = singles.tile([P, NB], F16, name="dmy")
        nc.vector.memset(dmy[:], 0.0)

        # ---- input DMAs, all issued upfront in strict need order.
        # x + w2 ride the HWDGE scalar queue; w1/w3 stream on the SWDGE
        # gpsimd queue (back-to-back, no ring stalls): w1 fb0, w3 fb0,
        # w1 fb1, w3 fb1.
        # Dependency tracking is tile-granular for these strided DMA
        # writes, so buffers are split so no consumer ever waits on a write
        # it doesn't need: x and w1-fb0 are split per chunk (sweep 1
        # consumes them progressively as they arrive); w3-fb0 / w1-fb1 /
        # w3-fb1 / w2 each get one tile per stream (their sweeps start
        # after the whole stream has landed anyway).
        CHUNKS = ((0, 2), (2, 2), (4, 4), (8, 4), (12, 4))
        xT_r = xT_d.rearrange("(k p) c -> p k c", p=P)
        w1_r = w1_d.rearrange("(k p) f -> p k f", p=P)
        w3_r = w3_d.rearrange("(k p) f -> p k f", p=P)
        xa = []
        for ci, (k0, kn) in enumerate(CHUNKS):
            xq = xpool.tile([P, kn, C], F16, name=f"xa{ci}")
            xa.append(xq)
            nc.scalar.dma_start(xq[:], xT_r[:, bass.ds(k0, kn), :])

        def xs(k, t):
            for ci, (k0, kn) in enumerate(CHUNKS):
                if k < k0 + kn:
                    return xa[ci][:, k - k0, t * P : t * P + P]

        # gpsimd FIFO in strict need order: w1 fb0 (per-chunk tiles),
        # w3 fb0, w1 fb1, w3 fb1, w2.
        w1a = []
        for ci, (k0, kn) in enumerate(CHUNKS):
            wt = w1pool.tile([P, kn, NB], F16, name=f"w1a{ci}")
            w1a.append(wt)
            nc.gpsimd.dma_start(wt[:], w1_r[:, bass.ds(k0, kn), 0:NB])
        w3a = w3pool.tile([P, KD, NB], F16, name="w3a")
        for q in range(2):
            ks = bass.ds(q * 8, 8)
            nc.gpsimd.dma_start(w3a[:, ks, :], w3_r[:, ks, 0:NB])
        w1b = w1pool.tile([P, KD, NB], F16, name="w1b")
        for q in range(2):
            ks = bass.ds(q * 8, 8)
            nc.gpsimd.dma_start(w1b[:, ks, :], w1_r[:, ks, NB : 2 * NB])
        w3b = w3pool.tile([P, KD, NB], F16, name="w3b")
        for q in range(2):
            ks = bass.ds(q * 8, 8)
            nc.gpsimd.dma_start(w3b[:, ks, :], w3_r[:, ks, NB : 2 * NB])

        def wsl(wi, fb, k):
            if wi == 0 and fb == 0:
                for ci, (k0, kn) in enumerate(CHUNKS):
                    if k < k0 + kn:
                        return w1a[ci][:, k - k0, :]
            ws = {(1, 0): w3a, (0, 1): w1b, (1, 1): w3b}[(wi, fb)]
            return ws[:, k, :]

        # warm up the PE p-state while the first x/w1 chunks are still in
        # flight: dependency-free dummy matmuls keep the array busy from
        # right after the framework preamble, so the real sweep starts at
        # full speed instead of ramping through the slow p-states.
        pdmy = ps_o.tile([P, NB], F32, tag="po", name="pdmy")
        for _ in range(11):
            nc.tensor.matmul(pdmy[:], dmy[:, 0:P], dmy[:], start=True, stop=True)

        w2s = w2pool.tile([P, KF, D], F16)
        w2_r = w2_d.rearrange("(k p) d -> p k d", p=P)
        for q in range(KF // 2):
            ks = bass.ds(q * 2, 2)
            nc.gpsimd.dma_start(w2s[:, ks, :], w2_r[:, ks, :])

        ssqa = stat.tile([P, NT], F32, name="ssqa")
        ssqb = stat.tile([P, NT], F32, name="ssqb")
        ssq = stat.tile([P, NT], F32, name="ssq")
        std = stat.tile([P, NT], F32, name="std")
        rstd = stat.tile([P, NT], F32, name="rstd")
        h = {
            t: hpool.tile([P, F], F16, tag=f"h{t}", name=f"h{t}")
            for t in range(NT)
        }
        ht = {
            t: htpool.tile([P, KF, P], F16, tag=f"ht{t}", name=f"ht{t}")
            for t in range(NT)
        }
        s = {}
        KQ = 4  # k-chunks per sweep quarter

        # ===== phase A: four sweeps (w1 fb0, w3 fb0, w1 fb1, w3 fb1).
        # Sweep 1 is k-outer so the PE consumes the just-arriving x/w1
        # chunks progressively; later sweeps have resident weights and run
        # t-outer so each tile's epilogue overlaps the next tile's matmuls.
        # Sweep 1 is k-outer so the PE consumes the just-arriving x/w1
        # chunks progressively; later sweeps have resident weights and run
        # t-outer so each tile's epilogue overlaps the next tile's matmuls.
        def sweep(wi, fb, epilogue, k_outer=False):
            ps = {}
            for t in range(NT):
                ps[t] = ps_a.tile(
                    [P, NB], F32, tag=f"pa{t}", name=f"ps_{wi}_{fb}_{t}"
                )
            if k_outer:
                for k in range(KD):
                    for t in range(NT):
                        nc.tensor.matmul(
                            ps[t][:],
                            xs(k, t),
                            wsl(wi, fb, k),
                            start=(k == 0),
                            stop=(k == KD - 1),
                        )
                for t in range(NT):
                    epilogue(t, ps[t])
            else:
                for t in range(NT):
                    for k in range(KD):
                        nc.tensor.matmul(
                            ps[t][:],
                            xs(k, t),
                            wsl(wi, fb, k),
                            start=(k == 0),
                            stop=(k == KD - 1),
                        )
                    epilogue(t, ps[t])

        def ep_silu1(t, ps):
            s[t] = spool.tile([P, NB], F32, tag=f"s{t}", name=f"s0_{t}")
            nc.scalar.activation(s[t][:], ps[:], ACTF.Silu)

        def ep_h0(t, ps):
            nc.vector.tensor_mul(h[t][:, 0:NB], s[t][:], ps[:])
            hsq = qpool.tile([P, NB], F16, tag="hsq", name=f"hsq0_{t}")
            nc.scalar.activation(
                hsq[:], h[t][:, 0:NB], ACTF.Square, accum_out=ssqa[:, t : t + 1]
            )
            nc.scalar.dma_start_transpose(
                ht[t][:, 0 : KF // 2, :], h[t][:, 0:NB]
            )

        def ep_silu2(t, ps):
            sn = spool.tile([P, NB], F32, tag=f"s{t}", name=f"s1_{t}")
            nc.scalar.activation(sn[:], ps[:], ACTF.Silu)
            s[t] = sn

        def ep_h1(t, ps):
            nc.vector.tensor_mul(h[t][:, NB : 2 * NB], s[t][:], ps[:])
            hsq = qpool.tile([P, NB], F16, tag="hsq", name=f"hsq1_{t}")
            nc.scalar.activation(
                hsq[:],
                h[t][:, NB : 2 * NB],
                ACTF.Square,
                accum_out=ssqb[:, t : t + 1],
            )
            nc.scalar.dma_start_transpose(
                ht[t][:, KF // 2 : KF, :], h[t][:, NB : 2 * NB]
            )
            nc.vector.tensor_add(
                ssq[:, t : t + 1], ssqa[:, t : t + 1], ssqb[:, t : t + 1]
            )
            nc.scalar.activation(
                std[:, t : t + 1],
                ssq[:, t : t + 1],
                ACTF.Sqrt,
                bias=eps_t[:],
                scale=1.0 / F,
            )
            nc.vector.reciprocal(rstd[:, t : t + 1], std[:, t : t + 1])

        sweep(0, 0, ep_silu1, k_outer=True)
        sweep(1, 0, ep_h0)
        sweep(0, 1, ep_silu2)
        sweep(1, 1, ep_h1)

        # ===== phase C: out = hT.T @ w2T, scaled by rstd.  db outer so the
        # w2 stream is consumed progressively; out DMAs per (t, db) chunk.
        for db in range(DB):
            dsl = bass.ds(db * NB, NB)
            for t in range(NT):
                pso = ps_o.tile([P, NB], F32, tag="po")
                for fc in range(KF):
                    nc.tensor.matmul(
                        pso[:],
                        ht[t][:, fc, :],
                        w2s[:, fc, dsl],
                        start=(fc == 0),
                        stop=(fc == KF - 1),
                    )
                ob = opool.tile([P, NB], F16, tag="ob", name=f"ob{t}_{db}")
                nc.vector.tensor_scalar_mul(ob[:], pso[:], rstd[:, t : t + 1])
                nc.sync.dma_start(out_d[t * P : (t + 1) * P, dsl], ob[:])

    nc.compile()
    return nc


def _get_program(C: int):
    if C not in _PROGRAM_CACHE:
        _PROGRAM_CACHE[C] = _build_program(C)
    return _PROGRAM_CACHE[C]


def kernel(x, w1, w2, w3, mid_w, num_tokens_per_expert):
    global LAST_RESULTS
    x = np.ascontiguousarray(np.asarray(x, dtype=np.float32))
    w1 = np.asarray(w1, dtype=np.float32)
    w2 = np.asarray(w2, dtype=np.float32)
    w3 = np.asarray(w3, dtype=np.float32)
    mid_w = np.asarray(mid_w, dtype=np.float32)
    counts = np.asarray(num_tokens_per_expert).astype(np.int64)

    T_, D_ = x.shape
    E_, F_, _ = w1.shape
    Ccap = (T_ // E_) * 3 // 2  # reference static capacity (768)
    ends = np.cumsum(counts)
    starts = ends - counts
    eff = np.minimum(np.maximum(counts, 0), Ccap)  # rows actually computed

    C = int(max(P, -(-int(eff.max()) // P) * P))  # pad to token-tile multiple
    nc = _get_program(C)

    in_maps = []
    for e in range(E_):
        cnt = int(eff[e])
        s = int(starts[e])
        xg = np.zeros((C, D_), np.float32)
        if cnt > 0:
            rows = np.clip(s + np.arange(cnt), 0, T_ - 1)
            xg[:cnt] = x[rows]
        in_maps.append(
            {
                "xT": np.ascontiguousarray(xg.T).astype(np.float16),
                "w1t": np.ascontiguousarray(w1[e].T).astype(np.float16),
                "w3t": np.ascontiguousarray(w3[e].T).astype(np.float16),
                "w2t": np.ascontiguousarray((w2[e] * mid_w[None, :]).T).astype(
                    np.float16
                ),
            }
        )

    LAST_RESULTS = _run(nc, in_maps)
    outs = [LAST_RESULTS[e]["out"] for e in range(E_)]

    # scatter back to flat token order, mirroring the reference's clamping
    tok = np.arange(T_)
    eid = np.clip(np.searchsorted(ends, tok, side="right"), 0, E_ - 1)
    pos = tok - starts[eid]
    idx = np.minimum(pos, Ccap - 1)
    valid = (idx >= 0) & (idx < eff[eid])
    idx_safe = np.clip(idx, 0, C - 1)
    stacked = np.stack(outs, axis=0)  # [E, C, D]
    result = stacked[eid, idx_safe].astype(np.float32)
    result[~valid] = 0.0
    return result
